# revision 5
# baseline (speedup 1.0000x reference)
"""Channel-transformer (CTR) attention kernel for Trainium2, 8 NeuronCores.

Problem: x (16, 256, 64, 64) f32, gamma scalar.
  xr = x.reshape(B, C, NH, DIM)                       # NH=8, DIM=512
  energy[b,h,c,k] = sum_d xr[b,c,h,d] * xr[b,k,h,d]   # symmetric (C x C)
  attn = softmax(rowmax(energy) - energy, axis=-1)    # == softmax(-energy)
  out[b,c,h,d] = sum_k attn[b,h,c,k] * xr[b,k,h,d]
  result = gamma * out + x

Sharding: data-parallel over batch, 2 samples per core; gamma replicated.

Per-core kernel (per batch b, head h):
  - keep x[b] resident in SBUF as two natural tiles X[m] = [128 ch, 4096]
  - XT (d-major) via 8 PE transposes of 128x128 blocks (f32)
  - E[m] = XT[:,m-half].T @ XT  (f32r matmuls, N=256, fp32 PSUM accumulate)
  - attnT[kc] = exp(-E[kc] - 64): the softmax max-shift cancels row-wise, so a
    constant bias suffices for range safety; E symmetric => E tiles are already
    the transposed-attention (k-major) layout the second matmul needs.
  - V[m] = sum_kc attnT[kc][:, m-half].T @ X[kc][:, head] (f32r, N=512)
    Z[m] = same weights against a ones column (row sums of unnormalized attn)
  - Y[m][:, head] = V[m] * (gamma / Z[m]) + X[m][:, head]  (one fused DVE op)
"""

import numpy as np

B, C, HW = 16, 256, 4096
NH, DIM = 8, 512
N_CORES = 8
BPC = B // N_CORES  # batches per core
EXP_BIAS = -64.0  # exp(-E + EXP_BIAS): keeps exponents < ~40 for N(0,1) inputs

_CACHE = {}


def _build_module():
    import concourse.bacc as bacc
    import concourse.tile as tile
    import concourse.mybir as mybir

    f32 = mybir.dt.float32
    f32r = mybir.dt.float32r
    AF = mybir.ActivationFunctionType
    OP = mybir.AluOpType

    nc = bacc.Bacc("TRN2", target_bir_lowering=False, debug=False, num_devices=N_CORES)
    x_d = nc.dram_tensor("x", [BPC, C, HW], f32r, kind="ExternalInput").ap()
    g_d = nc.dram_tensor("g", [1, 1], f32, kind="ExternalInput").ap()
    id_d = nc.dram_tensor("ident", [128, 128], f32r, kind="ExternalInput").ap()
    y_d = nc.dram_tensor("y", [BPC, C, HW], f32, kind="ExternalOutput").ap()

    with tile.TileContext(nc) as tc:
        from contextlib import ExitStack

        with ExitStack() as ctx:
            cpool = ctx.enter_context(tc.tile_pool(name="const", bufs=1))
            ident = cpool.tile([128, 128], f32r)
            nc.sync.dma_start(ident[:], id_d[:])
            ebias = cpool.tile([128, 1], f32)
            nc.gpsimd.memset(ebias[:], EXP_BIAS)
            ones = cpool.tile([128, 1], f32)
            nc.gpsimd.memset(ones[:], 1.0)
            onesr = cpool.tile([1, 128], f32)
            nc.gpsimd.memset(onesr[:], 1.0)
            gsb = cpool.tile([1, 1], f32)
            nc.sync.dma_start(gsb[:], g_d[:])
            gamma128 = cpool.tile([128, 1], f32)
            # broadcast gamma to all partitions: [128,1] = ones[1,128].T @ g[1,1]
            with tc.tile_pool(name="gps", bufs=1, space="PSUM") as gp:
                gps = gp.tile([128, 1], f32)
                nc.tensor.matmul(gps[:], onesr[:], gsb[:], start=True, stop=True)
                nc.scalar.copy(gamma128[:], gps[:])

            x_pool = ctx.enter_context(tc.tile_pool(name="xs", bufs=2 * BPC))
            y_pool = ctx.enter_context(tc.tile_pool(name="ys", bufs=2 * BPC))
            xt_pool = ctx.enter_context(tc.tile_pool(name="xt", bufs=8))
            at_pool = ctx.enter_context(tc.tile_pool(name="at", bufs=4))
            r_pool = ctx.enter_context(tc.tile_pool(name="rp", bufs=8))
            tp_pool = ctx.enter_context(tc.tile_pool(name="tp", bufs=2, space="PSUM"))
            e_pool = ctx.enter_context(tc.tile_pool(name="pe", bufs=2, space="PSUM"))
            v_pool = ctx.enter_context(tc.tile_pool(name="pv", bufs=4, space="PSUM"))

            for b in range(BPC):
                X = [x_pool.tile([128, HW], f32r, tag="xs", name=f"X{b}_{m}") for m in range(2)]
                for m in range(2):
                    nc.sync.dma_start(X[m][:], x_d[b, 128 * m : 128 * (m + 1), :])
                Y = [y_pool.tile([128, HW], f32, tag="ys", name=f"Y{b}_{m}") for m in range(2)]

                for h in range(NH):
                    col = DIM * h
                    # ---- XT[kd] = [128 d, 256 ch] for the 4 d-chunks ----
                    XT = []
                    for kd in range(4):
                        t = xt_pool.tile([128, 256], f32r, tag="xt", name=f"XT{b}_{h}_{kd}")
                        for m in range(2):
                            tp = tp_pool.tile([128, 128], f32r, tag="tp", name=f"TP{b}_{h}_{kd}_{m}")
                            nc.tensor.transpose(
                                tp[:],
                                X[m][:, col + 128 * kd : col + 128 * (kd + 1)],
                                ident[:],
                            )
                            nc.scalar.copy(t[:, 128 * m : 128 * (m + 1)], tp[:])
                        XT.append(t)

                    # ---- E[m] = XT[:, m-half].T @ XT  (accumulate over kd) ----
                    E = [e_pool.tile([128, 256], f32, tag="pe", name=f"E{b}_{h}_{m}") for m in range(2)]
                    for m in range(2):
                        for kd in range(4):
                            nc.tensor.matmul(
                                E[m][:],
                                XT[kd][:, 128 * m : 128 * (m + 1)],
                                XT[kd][:],
                                start=(kd == 0),
                                stop=(kd == 3),
                            )

                    # ---- attnT[kc] = exp(-E - 64); E symmetric, so this tile is
                    # unnormalized-attn^T with k on partitions ----
                    AT = []
                    for kc in range(2):
                        a = at_pool.tile([128, 256], f32r, tag="at", name=f"AT{b}_{h}_{kc}")
                        nc.scalar.activation(
                            a[:], E[kc][:], AF.Exp, scale=-1.0, bias=ebias[:]
                        )
                        AT.append(a)

                    # ---- V[m] += attnT[kc][:, m-half].T @ X[kc][:, head]
                    #      Z[m] += same weights @ ones  (row sums) ----
                    V = [v_pool.tile([128, DIM], f32, tag="pv", name=f"V{b}_{h}_{m}") for m in range(2)]
                    Z = [e_pool.tile([128, 1], f32, tag="pe", name=f"Z{b}_{h}_{m}") for m in range(2)]
                    for m in range(2):
                        for kc in range(2):
                            w = AT[kc][:, 128 * m : 128 * (m + 1)]
                            nc.tensor.matmul(
                                V[m][:],
                                w,
                                X[kc][:, col : col + DIM],
                                start=(kc == 0),
                                stop=(kc == 1),
                            )
                            nc.tensor.matmul(
                                Z[m][:],
                                w.bitcast(f32),
                                ones[:],
                                start=(kc == 0),
                                stop=(kc == 1),
                            )

                    # ---- Y[m][:, head] = V * (gamma / Z) + X[m][:, head] ----
                    for m in range(2):
                        R = r_pool.tile([128, 1], f32, tag="rp", name=f"R{b}_{h}_{m}")
                        nc.vector.reciprocal(R[:], Z[m][:])
                        gR = r_pool.tile([128, 1], f32, tag="rp", name=f"R{b}_{h}_{m}")
                        nc.vector.tensor_tensor(gR[:], R[:], gamma128[:], op=OP.mult)
                        nc.vector.scalar_tensor_tensor(
                            Y[m][:, col : col + DIM],
                            V[m][:],
                            gR[:],
                            X[m][:, col : col + DIM].bitcast(f32),
                            op0=OP.mult,
                            op1=OP.add,
                        )

                for m in range(2):
                    nc.sync.dma_start(y_d[b, 128 * m : 128 * (m + 1), :], Y[m][:])

    nc.compile()
    return nc


def _get_module():
    if "nc" not in _CACHE:
        _CACHE["nc"] = _build_module()
    return _CACHE["nc"]


def kernel(x_input, gamma):
    from concourse.bass_utils import run_bass_kernel_spmd

    nc = _get_module()

    x = np.ascontiguousarray(np.asarray(x_input, dtype=np.float32)).reshape(B, C, HW)
    g = np.asarray(gamma, dtype=np.float32).reshape(1, 1)
    ident = np.eye(128, dtype=np.float32)

    in_maps = [
        {
            "x": np.ascontiguousarray(x[i * BPC : (i + 1) * BPC]),
            "g": g,
            "ident": ident,
        }
        for i in range(N_CORES)
    ]
    res = run_bass_kernel_spmd(nc, in_maps, list(range(N_CORES)))
    y = np.concatenate([res.results[i]["y"] for i in range(N_CORES)], axis=0)
    return y.reshape(B, C, 64, 64).astype(np.float32)
